# revision 2
# baseline (speedup 1.0000x reference)
"""CapsEEGNet kernel for 8 Trainium2 NeuronCores.

Pure data parallel over batch B=256 -> 8 shards of 32 (weights
replicated). One jit-compiled SPMD program over a 1-D device mesh; the
per-shard computation is expressed as matmul/einsum-friendly ops
(shift-stacked convolutions) so it maps onto the TensorEngine.
"""
import numpy as np
import jax
import jax.numpy as jnp
from jax.sharding import Mesh, NamedSharding, PartitionSpec as P

EPS = 1e-7
ROUTINGS = 3
N_CORES = 8

_STATE = None


def _squash(x):
    sq = jnp.sum(x * x + EPS, axis=-1, keepdims=True)
    return sq * x / ((1.0 + sq) * jnp.sqrt(sq))


def _forward(x, conv1_w, bn1_g, bn1_b, bn1_m, bn1_v, dw_w,
             bn2_g, bn2_b, bn2_m, bn2_v, pc_w, pc_b, pc2_w, pc2_b,
             em_W, fc_w, fc_b):
    B = x.shape[0]
    Chans, S = x.shape[2], x.shape[3]

    # ---- conv1: 1D conv along s (taps 64, 'same' pad 31/32) + bn1 + elu
    # fold bn1 into the conv weight/bias
    inv1 = bn1_g / jnp.sqrt(bn1_v + 1e-5)
    w1 = conv1_w[:, 0, 0, :] * inv1[:, None]            # (8, 64)
    b1 = bn1_b - bn1_m * inv1                           # (8,)
    xs = x[:, 0]                                        # (B, 32, 128)
    xpad = jnp.pad(xs, ((0, 0), (0, 0), (31, 32)))      # (B, 32, 191)
    # windows: (B, 32, 128, 64) -- 64 shifted views
    Xw = jnp.stack([xpad[:, :, t:t + S] for t in range(64)], axis=-1)
    h1 = jnp.einsum('bcst,ot->bocs', Xw, w1) + b1[None, :, None, None]
    h1 = jax.nn.elu(h1)                                 # (B, 8, 32, 128)

    # ---- constrained depthwise conv over chans (groups=8, 2 out per group)
    norm = jnp.sqrt(jnp.sum(dw_w ** 2, axis=(1, 2, 3), keepdims=True))
    w = dw_w * jnp.where(norm > 1.0, 1.0 / (norm + 1e-7), 1.0)
    wg = w[:, 0, :, 0].reshape(8, 2, Chans)             # (8 groups, 2, 32)
    inv2 = bn2_g / jnp.sqrt(bn2_v + 1e-5)
    b2 = bn2_b - bn2_m * inv2
    h2 = jnp.einsum('bgcs,goc->bgos', h1, wg).reshape(B, 16, S)
    h2 = h2 * inv2[None, :, None] + b2[None, :, None]
    h2 = jax.nn.elu(h2)                                 # (B, 16, 128)

    # ---- PrimaryCap conv (taps 6, pad 2/3) + bias
    h2p = jnp.pad(h2, ((0, 0), (0, 0), (2, 3)))         # (B, 16, 133)
    Hw = jnp.stack([h2p[:, :, t:t + S] for t in range(6)], axis=-1)
    pcw = pc_w[:, :, 0, :]                              # (256, 16, 6)
    out = jnp.einsum('bcst,pct->bps', Hw, pcw) + pc_b[None, :, None]

    # ---- concat + 1x1 conv
    cat = jnp.concatenate([h2, out], axis=1)            # (B, 272, 128)
    w2 = pc2_w[:, :, 0, 0]                              # (256, 272)
    out = jnp.einsum('bcs,pc->bps', cat, w2) + pc2_b[None, :, None]

    # ---- squash into capsules
    u = _squash(out.reshape(B, -1, 8))                  # (B, 4096, 8)

    # ---- EmotionCap dynamic routing
    u_hat = jnp.einsum('kndi,bni->bknd', em_W, u)       # (B, 4, 4096, 16)
    b = jnp.zeros(u_hat.shape[:3], u_hat.dtype)
    for i in range(ROUTINGS):
        c = jax.nn.softmax(b, axis=1)
        v = _squash(jnp.einsum('bkn,bknd->bkd', c, u_hat))
        if i < ROUTINGS - 1:
            b = b + jnp.einsum('bknd,bkd->bkn', u_hat, v)
    logits = jnp.einsum('bkd,od->bko', v, fc_w)[..., 0] + fc_b[0]
    return jax.nn.softmax(logits, axis=1)


def _get_state():
    global _STATE
    if _STATE is None:
        devs = np.array(jax.devices()[:N_CORES])
        mesh = Mesh(devs, ('b',))
        sh_b = NamedSharding(mesh, P('b'))
        sh_r = NamedSharding(mesh, P())
        wnames = ['conv1_w', 'bn1_g', 'bn1_b', 'bn1_m', 'bn1_v', 'dw_w',
                  'bn2_g', 'bn2_b', 'bn2_m', 'bn2_v', 'pc_w', 'pc_b',
                  'pc2_w', 'pc2_b', 'em_W', 'fc_w', 'fc_b']
        in_sh = tuple([sh_b] + [sh_r] * len(wnames))
        fn = jax.jit(_forward, in_shardings=in_sh, out_shardings=sh_b)
        _STATE = (mesh, sh_b, sh_r, wnames, fn)
    return _STATE


def kernel(**inputs) -> np.ndarray:
    mesh, sh_b, sh_r, wnames, fn = _get_state()
    x = jax.device_put(np.asarray(inputs['x'], np.float32), sh_b)
    ws = [jax.device_put(np.asarray(inputs[k], np.float32), sh_r)
          for k in wnames]
    out = fn(x, *ws)
    return np.asarray(out).astype(np.float32)


if __name__ == '__main__':
    import reference
    inp = {k: np.asarray(v) for k, v in reference.setup_inputs().items()}
    got = kernel(**inp)
    print("out shape", got.shape, got.dtype)


# revision 3
# speedup vs baseline: 8.6633x; 8.6633x over previous
"""CapsEEGNet kernel for 8 Trainium2 NeuronCores.

Pure data parallel over batch B=256 -> 8 shards of 32 (weights
replicated). One jit-compiled SPMD program over a 1-D device mesh; the
per-shard computation is expressed as matmul/einsum-friendly ops
(shift-stacked convolutions) so it maps onto the TensorEngine.
"""
import numpy as np
import jax
import jax.numpy as jnp
from jax.sharding import Mesh, NamedSharding, PartitionSpec as P

EPS = 1e-7
ROUTINGS = 3
N_CORES = 8

_STATE = None


def _squash(x):
    sq = jnp.sum(x * x + EPS, axis=-1, keepdims=True)
    return sq * x / ((1.0 + sq) * jnp.sqrt(sq))


def _forward(x, conv1_w, bn1_g, bn1_b, bn1_m, bn1_v, dw_w,
             bn2_g, bn2_b, bn2_m, bn2_v, pc_w, pc_b, pc2_w, pc2_b,
             em_W, fc_w, fc_b):
    B = x.shape[0]
    Chans, S = x.shape[2], x.shape[3]

    # ---- conv1: 1D conv along s (taps 64, 'same' pad 31/32) + bn1 + elu
    # fold bn1 into the conv weight/bias
    inv1 = bn1_g / jnp.sqrt(bn1_v + 1e-5)
    w1 = conv1_w[:, 0, 0, :] * inv1[:, None]            # (8, 64)
    b1 = bn1_b - bn1_m * inv1                           # (8,)
    xs = x[:, 0]                                        # (B, 32, 128)
    xpad = jnp.pad(xs, ((0, 0), (0, 0), (31, 32)))      # (B, 32, 191)
    # windows: (B, 32, 128, 64) -- 64 shifted views
    Xw = jnp.stack([xpad[:, :, t:t + S] for t in range(64)], axis=-1)
    h1 = jnp.einsum('bcst,ot->bocs', Xw, w1) + b1[None, :, None, None]
    h1 = jax.nn.elu(h1)                                 # (B, 8, 32, 128)

    # ---- constrained depthwise conv over chans (groups=8, 2 out per group)
    norm = jnp.sqrt(jnp.sum(dw_w ** 2, axis=(1, 2, 3), keepdims=True))
    w = dw_w * jnp.where(norm > 1.0, 1.0 / (norm + 1e-7), 1.0)
    wg = w[:, 0, :, 0].reshape(8, 2, Chans)             # (8 groups, 2, 32)
    inv2 = bn2_g / jnp.sqrt(bn2_v + 1e-5)
    b2 = bn2_b - bn2_m * inv2
    h2 = jnp.einsum('bgcs,goc->bgos', h1, wg).reshape(B, 16, S)
    h2 = h2 * inv2[None, :, None] + b2[None, :, None]
    h2 = jax.nn.elu(h2)                                 # (B, 16, 128)

    # ---- PrimaryCap conv (taps 6, pad 2/3) + bias
    h2p = jnp.pad(h2, ((0, 0), (0, 0), (2, 3)))         # (B, 16, 133)
    Hw = jnp.stack([h2p[:, :, t:t + S] for t in range(6)], axis=-1)
    pcw = pc_w[:, :, 0, :]                              # (256, 16, 6)
    out = jnp.einsum('bcst,pct->bps', Hw, pcw) + pc_b[None, :, None]

    # ---- concat + 1x1 conv
    cat = jnp.concatenate([h2, out], axis=1)            # (B, 272, 128)
    w2 = pc2_w[:, :, 0, 0]                              # (256, 272)
    out = jnp.einsum('bcs,pc->bps', cat, w2) + pc2_b[None, :, None]

    # ---- squash into capsules
    u = _squash(out.reshape(B, -1, 8))                  # (B, 4096, 8)

    # ---- EmotionCap dynamic routing
    u_hat = jnp.einsum('kndi,bni->bknd', em_W, u)       # (B, 4, 4096, 16)
    b = jnp.zeros(u_hat.shape[:3], u_hat.dtype)
    for i in range(ROUTINGS):
        c = jax.nn.softmax(b, axis=1)
        v = _squash(jnp.einsum('bkn,bknd->bkd', c, u_hat))
        if i < ROUTINGS - 1:
            b = b + jnp.einsum('bknd,bkd->bkn', u_hat, v)
    logits = jnp.einsum('bkd,od->bko', v, fc_w)[..., 0] + fc_b[0]
    return jax.nn.softmax(logits, axis=1)


def _get_state():
    global _STATE
    if _STATE is None:
        devs = np.array(jax.devices()[:N_CORES])
        mesh = Mesh(devs, ('b',))
        sh_b = NamedSharding(mesh, P('b'))
        sh_r = NamedSharding(mesh, P())
        wnames = ['conv1_w', 'bn1_g', 'bn1_b', 'bn1_m', 'bn1_v', 'dw_w',
                  'bn2_g', 'bn2_b', 'bn2_m', 'bn2_v', 'pc_w', 'pc_b',
                  'pc2_w', 'pc2_b', 'em_W', 'fc_w', 'fc_b']
        in_sh = tuple([sh_b] + [sh_r] * len(wnames))
        fn = jax.jit(_forward, in_shardings=in_sh, out_shardings=sh_b)
        _STATE = (mesh, sh_b, sh_r, wnames, fn)
    return _STATE


_WCACHE = {'key': None, 'ws': None}


def _weight_key(inputs, wnames):
    h = 0
    for k in wnames:
        a = np.asarray(inputs[k])
        h ^= hash((k, a.shape, a.dtype.str, a.tobytes()[:256]))
    return h


def kernel(**inputs) -> np.ndarray:
    mesh, sh_b, sh_r, wnames, fn = _get_state()
    x = jax.device_put(np.asarray(inputs['x'], np.float32), sh_b)
    key = _weight_key(inputs, wnames)
    if _WCACHE['key'] != key:
        _WCACHE['ws'] = [
            jax.device_put(np.asarray(inputs[k], np.float32), sh_r)
            for k in wnames]
        _WCACHE['key'] = key
    out = fn(x, *_WCACHE['ws'])
    return np.asarray(out).astype(np.float32)


if __name__ == '__main__':
    import reference
    inp = {k: np.asarray(v) for k, v in reference.setup_inputs().items()}
    got = kernel(**inp)
    print("out shape", got.shape, got.dtype)


# revision 4
# speedup vs baseline: 8.7891x; 1.0145x over previous
"""CapsEEGNet kernel for 8 Trainium2 NeuronCores.

Pure data parallel over batch B=256 -> 8 shards of 32 (weights
replicated). One jit-compiled SPMD program over a 1-D device mesh; the
per-shard computation is expressed as matmul/einsum-friendly ops
(shift-stacked convolutions) so it maps onto the TensorEngine.
"""
import numpy as np
import jax
import jax.numpy as jnp
from jax.sharding import Mesh, NamedSharding, PartitionSpec as P

EPS = 1e-7
ROUTINGS = 3
N_CORES = 8

_STATE = None


def _squash(x):
    sq = jnp.sum(x * x + EPS, axis=-1, keepdims=True)
    return sq * x / ((1.0 + sq) * jnp.sqrt(sq))


def _forward(x, conv1_w, bn1_g, bn1_b, bn1_m, bn1_v, dw_w,
             bn2_g, bn2_b, bn2_m, bn2_v, pc_w, pc_b, pc2_w, pc2_b,
             em_W, fc_w, fc_b):
    B = x.shape[0]
    Chans, S = x.shape[2], x.shape[3]

    # ---- conv1: 1D conv along s (taps 64, 'same' pad 31/32) + bn1 + elu
    # fold bn1 into the conv weight/bias
    inv1 = bn1_g / jnp.sqrt(bn1_v + 1e-5)
    w1 = conv1_w[:, 0, 0, :] * inv1[:, None]            # (8, 64)
    b1 = bn1_b - bn1_m * inv1                           # (8,)
    xs = x[:, 0]                                        # (B, 32, 128)
    xpad = jnp.pad(xs, ((0, 0), (0, 0), (31, 32)))      # (B, 32, 191)
    # windows: (B, 32, 128, 64) -- 64 shifted views
    Xw = jnp.stack([xpad[:, :, t:t + S] for t in range(64)], axis=-1)
    h1 = jnp.einsum('bcst,ot->bocs', Xw, w1) + b1[None, :, None, None]
    h1 = jax.nn.elu(h1)                                 # (B, 8, 32, 128)

    # ---- constrained depthwise conv over chans (groups=8, 2 out per group)
    norm = jnp.sqrt(jnp.sum(dw_w ** 2, axis=(1, 2, 3), keepdims=True))
    w = dw_w * jnp.where(norm > 1.0, 1.0 / (norm + 1e-7), 1.0)
    wg = w[:, 0, :, 0].reshape(8, 2, Chans)             # (8 groups, 2, 32)
    inv2 = bn2_g / jnp.sqrt(bn2_v + 1e-5)
    b2 = bn2_b - bn2_m * inv2
    h2 = jnp.einsum('bgcs,goc->bgos', h1, wg).reshape(B, 16, S)
    h2 = h2 * inv2[None, :, None] + b2[None, :, None]
    h2 = jax.nn.elu(h2)                                 # (B, 16, 128)

    # ---- PrimaryCap conv (taps 6, pad 2/3) + bias
    h2p = jnp.pad(h2, ((0, 0), (0, 0), (2, 3)))         # (B, 16, 133)
    Hw = jnp.stack([h2p[:, :, t:t + S] for t in range(6)], axis=-1)
    pcw = pc_w[:, :, 0, :]                              # (256, 16, 6)
    out = jnp.einsum('bcst,pct->bps', Hw, pcw) + pc_b[None, :, None]

    # ---- concat + 1x1 conv
    cat = jnp.concatenate([h2, out], axis=1)            # (B, 272, 128)
    w2 = pc2_w[:, :, 0, 0]                              # (256, 272)
    out = jnp.einsum('bcs,pc->bps', cat, w2) + pc2_b[None, :, None]

    # ---- squash into capsules
    u = _squash(out.reshape(B, -1, 8))                  # (B, 4096, 8)

    # ---- EmotionCap dynamic routing (u_hat never materialized):
    # u_hat[b,k,n,d] = sum_i em_W[k,n,d,i] u[b,n,i]
    # iter 1: c is uniform (b=0) -> s = 0.25 * sum_n u_hat, contracted
    # directly over (n,i) with no large intermediate.
    s = 0.25 * jnp.einsum('kndi,bni->bkd', em_W, u)
    v = _squash(s)
    rb = None
    for i in range(1, ROUTINGS):
        # b += sum_d u_hat*v  via g[b,k,n,i] = sum_d em_W*v  (16.8MB/shard)
        g = jnp.einsum('kndi,bkd->bkni', em_W, v)
        step = jnp.einsum('bkni,bni->bkn', g, u)
        rb = step if rb is None else rb + step
        c = jax.nn.softmax(rb, axis=1)
        # s = sum_n c*u_hat  via tc = c (x) u  (16.8MB/shard)
        tc = c[..., None] * u[:, None, :, :]
        s = jnp.einsum('kndi,bkni->bkd', em_W, tc)
        v = _squash(s)
    logits = jnp.einsum('bkd,od->bko', v, fc_w)[..., 0] + fc_b[0]
    return jax.nn.softmax(logits, axis=1)


def _get_state():
    global _STATE
    if _STATE is None:
        devs = np.array(jax.devices()[:N_CORES])
        mesh = Mesh(devs, ('b',))
        sh_b = NamedSharding(mesh, P('b'))
        sh_r = NamedSharding(mesh, P())
        wnames = ['conv1_w', 'bn1_g', 'bn1_b', 'bn1_m', 'bn1_v', 'dw_w',
                  'bn2_g', 'bn2_b', 'bn2_m', 'bn2_v', 'pc_w', 'pc_b',
                  'pc2_w', 'pc2_b', 'em_W', 'fc_w', 'fc_b']
        in_sh = tuple([sh_b] + [sh_r] * len(wnames))
        fn = jax.jit(_forward, in_shardings=in_sh, out_shardings=sh_b)
        _STATE = (mesh, sh_b, sh_r, wnames, fn)
    return _STATE


_WCACHE = {'key': None, 'ws': None}


def _weight_key(inputs, wnames):
    h = 0
    for k in wnames:
        a = np.asarray(inputs[k])
        h ^= hash((k, a.shape, a.dtype.str, a.tobytes()[:256]))
    return h


def kernel(**inputs) -> np.ndarray:
    mesh, sh_b, sh_r, wnames, fn = _get_state()
    x = jax.device_put(np.asarray(inputs['x'], np.float32), sh_b)
    key = _weight_key(inputs, wnames)
    if _WCACHE['key'] != key:
        _WCACHE['ws'] = [
            jax.device_put(np.asarray(inputs[k], np.float32), sh_r)
            for k in wnames]
        _WCACHE['key'] = key
    out = fn(x, *_WCACHE['ws'])
    return np.asarray(out).astype(np.float32)


if __name__ == '__main__':
    import reference
    inp = {k: np.asarray(v) for k, v in reference.setup_inputs().items()}
    got = kernel(**inp)
    print("out shape", got.shape, got.dtype)
